# revision 2
# baseline (speedup 1.0000x reference)
"""Gated max/avg 2x2 pooling kernel for Trainium2 (8 NeuronCores, SPMD) — v2.

Reference computation (per 2x2 window over [B, H, W, C], stride 2):
    x1 = max(window), x2 = mean(window)
    xs = sum_ij mask[i, j] * window[i, j]   (per channel)
    z  = sigmoid(xs)
    out = z * x1 + (1 - z) * x2

Engine split (measured rates: DVE TT 1.0u, TS 0.61u, STT/custom 2.0u;
ACT 1.7u + 440ns/instr; GPSIMD 3.3u):
  ACT : f32->f16 cast (1 instr/tile) + sigmoid
  DVE : max tree (2 TT), sum tree (2 TT), xs pair-Horner (3 TS + 3 TT),
        t = 0.25*s (TS), d = x1 - t (TT)
  GPS : g = z*d, o = t + g   (two light ops on the otherwise idle engine)
  DMA : f32 in, f16 out (host upcasts)

xs pair-Horner: A' = rE*Ea + Eb, B' = rO*Oa + Ob, xsp = rB*Bp + Ap,
z = sigmoid(f * xsp), with (a, b) per pair and the (Bp, Ap) order chosen
on the host so every ratio has |r| <= 1; f is the remaining carrier
scale, applied for free by the ACT sigmoid.
"""

import numpy as np

import concourse.bacc as bacc
import concourse.mybir as mybir
import concourse.tile as tile
from concourse.bass_utils import run_bass_kernel_spmd

F32 = mybir.dt.float32
F16 = mybir.dt.float16

B, H, W, C = 16, 256, 256, 64
N_CORES = 8
BPC = B // N_CORES          # batches per core
HO = H // 2                 # 128 output rows = SBUF partitions
NQ = 4                      # w-quarters per row
WQ = W // NQ                # input w per macro-tile (64)

LAST_EXEC_NS = None
LAST_RESULTS = None

_PROGRAM_CACHE = {}


def _build_program(bpc, ho, nq, wq, ch, plan):
    """Build + compile the single-core Bass/Tile program (SPMD-shared).

    plan = (swapE, swapO, swapF, rE, rO, rB): slice assignment for the
    pair-Horner plus the ratio values, baked as instruction immediates
    (PTR-scalar tensor_scalar ops stall behind concurrent GPSIMD work;
    immediate-scalar ones do not).
    """
    from contextlib import ExitStack

    assert ho == 128, "partition dim must be 128"
    swapE, swapO, swapF, rE, rO, rB = plan
    fd_in = wq * ch            # free dim of an E/O tile (4096)
    wo = wq // 2               # output w per macro-tile
    fd_out = wo * ch           # free dim of output tile (2048)

    nc = bacc.Bacc(
        "TRN2",
        target_bir_lowering=False,
        debug=False,
        enable_asserts=True,
        num_devices=N_CORES,
    )

    x = nc.dram_tensor("x", [bpc, ho, 2, nq, fd_in], F32, kind="ExternalInput")
    scal = nc.dram_tensor("scal", [128, 8], F32, kind="ExternalInput")
    out = nc.dram_tensor("out", [bpc, ho, nq, fd_out], F16, kind="ExternalOutput")
    x_ap = x.ap()
    out_ap = out.ap()

    alu = mybir.AluOpType

    with tile.TileContext(nc) as tc, ExitStack() as ctx:
        pool_io = ctx.enter_context(tc.tile_pool(name="io", bufs=2))
        pool_y = ctx.enter_context(tc.tile_pool(name="y16", bufs=2))
        pool_big = ctx.enter_context(tc.tile_pool(name="big", bufs=1))
        pool_tmp = ctx.enter_context(tc.tile_pool(name="tmp", bufs=1))
        pool_gps = ctx.enter_context(tc.tile_pool(name="gps", bufs=2))
        pool_out = ctx.enter_context(tc.tile_pool(name="outp", bufs=2))
        pool_const = ctx.enter_context(tc.tile_pool(name="const", bufs=1))

        scal_t = pool_const.tile([128, 8], F32)
        nc.sync.dma_start(scal_t[:], scal.ap()[:])
        f_ap = scal_t[:, 3:4]
        rB_ap = scal_t[:, 2:3]
        zero_ap = scal_t[:, 4:5]
        quart_ap = scal_t[:, 5:6]  # 0.25

        def emit_load(b, q, w_lo, w_hi):
            """Stage 1: input DMA + ACT cast for one tile."""
            nw = w_hi - w_lo
            fde = nw * 2 * ch
            EO = pool_io.tile([128, 2 * fde], F32, tag="EO")
            src = x_ap[b, :, :, q, :].rearrange(
                "p r (w c) -> p r w c", c=2 * ch
            )[:, :, w_lo : w_lo + nw, :]
            nc.sync.dma_start(
                EO[:].rearrange("p (r w c) -> p r w c", r=2, c=2 * ch), src
            )
            Y = pool_y.tile([128, 2 * fde], F16, tag="Y")
            # cast + column-deinterleave: write order (r, e, w, c) while
            # reading DMA order (r, w, e, c); downstream slices contiguous.
            # One cast per row-parity keeps the APs within 3 free dims.
            Yw = Y[:].rearrange(
                "p (r e w c) -> p r w e c", r=2, e=2, c=ch
            )
            EOr = EO[:].rearrange(
                "p (r w e c) -> p r w e c", r=2, e=2, c=ch)
            nc.scalar.copy(Yw[:, 0], EOr[:, 0])
            nc.scalar.copy(Yw[:, 1], EOr[:, 1])
            return dict(b=b, q=q, w_lo=w_lo, nw=nw, fde=fde, fdo=nw * ch, Y=Y)

        def emit_compute(h):
            """Stage 2: DVE trees + sigmoid + GPS blend + output DMA."""
            b, q, w_lo, nw = h["b"], h["q"], h["w_lo"], h["nw"]
            fde, fdo, Y = h["fde"], h["fdo"], h["Y"]

            def tmp(tag, fd=fdo, dt=F16, pool=pool_tmp):
                t = pool.tile([128, fd], dt, tag=tag)
                return t

            Ef = Y[:, 0:fde]
            Of = Y[:, fde : 2 * fde]
            # deinterleaved layout: all four window slices are contiguous
            Ee, Eo = Y[:, 0:fdo], Y[:, fdo : 2 * fdo]
            Oe, Oo = Y[:, fde : fde + fdo], Y[:, fde + fdo : 2 * fde]

            def v(t):
                return t[:].rearrange("p (w c) -> p w c", c=ch)

            # max tree (M1 inherits the (e, w, c) order: halves contiguous)
            M1 = tmp("M1", fd=fde, pool=pool_big)
            nc.vector.tensor_max(M1[:], Ef, Of)
            x1 = tmp("x1")
            nc.vector.tensor_max(x1[:], M1[:, 0:fdo], M1[:, fdo:fde])

            # sum tree
            S1 = tmp("S1", fd=fde, pool=pool_big)
            nc.vector.tensor_add(S1[:], Ef, Of)
            s = tmp("s", pool=pool_gps)
            nc.vector.tensor_add(s[:], S1[:, 0:fdo], S1[:, fdo:fde])
            t4 = tmp("t4", pool=pool_gps)
            nc.scalar.mul(t4[:], s[:], quart_ap)

            # xs pair-Horner
            Ea, Eb = (Eo, Ee) if swapE else (Ee, Eo)
            Oa, Ob = (Oo, Oe) if swapO else (Oe, Oo)
            q1 = tmp("q1")
            nc.vector.tensor_scalar_mul(q1[:], Ea, float(rE))
            Ap = tmp("Ap", pool=pool_gps)
            nc.vector.tensor_add(Ap[:], q1[:], Eb)
            q2 = tmp("q2")
            nc.vector.tensor_scalar_mul(q2[:], Oa, float(rO))
            Bp = tmp("Bp", pool=pool_gps)
            nc.vector.tensor_add(Bp[:], q2[:], Ob)
            # final combine: scale the weaker pair
            Fa, Fb = (Ap, Bp) if swapF else (Bp, Ap)
            q3 = tmp("q3", pool=pool_gps)
            nc.scalar.mul(q3[:], Fa[:], rB_ap)
            xsp = tmp("xsp", pool=pool_gps)
            nc.vector.tensor_add(xsp[:], q3[:], Fb[:])

            # z = sigmoid(f * xsp) on ACT
            z = tmp("z", pool=pool_gps)
            nc.scalar.activation(
                z[:], xsp[:],
                mybir.ActivationFunctionType.Sigmoid,
                bias=zero_ap, scale=f_ap,
            )

            # blend: d = x1 - t4; g = z*d; o = t4 + g
            d = tmp("d", pool=pool_gps)
            nc.vector.tensor_sub(d[:], x1[:], t4[:])
            g = tmp("g", pool=pool_gps)
            nc.vector.tensor_mul(g[:], z[:], d[:])
            o = tmp("o", pool=pool_out)
            nc.vector.tensor_add(o[:], t4[:], g[:])

            dst = out_ap[b, :, q, :].rearrange("p (w c) -> p w c", c=ch)
            nc.sync.dma_start(
                dst[:, w_lo : w_lo + nw, :],
                o[:].rearrange("p (w c) -> p w c", c=ch),
            )

        wo_q = wq // 2  # output w-pairs per quarter
        n_macro = bpc * nq
        tiles = []
        for b in range(bpc):
            for qq in range(nq):
                first = not tiles
                last = b == bpc - 1 and qq == nq - 1
                if first:
                    tiles.append((b, qq, 0, wo_q // 4))
                    tiles.append((b, qq, wo_q // 4, wo_q // 2))
                    tiles.append((b, qq, wo_q // 2, wo_q))
                elif last:
                    tiles.append((b, qq, 0, wo_q // 2))
                    tiles.append((b, qq, wo_q // 2, wo_q))
                else:
                    tiles.append((b, qq, 0, wo_q))
        pending = emit_load(*tiles[0])
        for i in range(len(tiles)):
            nxt = emit_load(*tiles[i + 1]) if i + 1 < len(tiles) else None
            emit_compute(pending)
            pending = nxt

    nc.compile()
    return nc


def _get_program(bpc, ho, nq, wq, ch, plan):
    key = (bpc, ho, nq, wq, ch, plan)
    if key not in _PROGRAM_CACHE:
        _PROGRAM_CACHE[key] = _build_program(bpc, ho, nq, wq, ch, plan)
    return _PROGRAM_CACHE[key]


def _mask_plan(mask):
    """Derive (plan, scal[128,8]) so every ratio has |r| <= 1.

    xs = m00*Ee + m01*Eo + m10*Oe + m11*Oo
       = cE*(rE*Ea + Eb) + cO*(rO*Oa + Ob)
    with (Ea, Eb) = (Ee, Eo) or swapped so |rE| <= 1 (cE = the larger-|.|
    coefficient of the E pair), likewise the O pair. Final:
    xsp = rB*Fa + Fb with Fb the pair whose carrier |c| is larger;
    f = carrier of Fb, rB = other carrier / f.
    """
    m = np.asarray(mask, np.float64).reshape(-1)  # m00, m01, m10, m11
    mE = (m[0], m[1])
    mO = (m[2], m[3])

    def pair(coeffs):
        ca, cb = coeffs  # coeff of the 'even' slice, coeff of the 'odd' slice
        # swapped=False: A' = r*even + odd, carrier = cb (odd coeff), r = ca/cb
        # swapped=True:  A' = r*odd + even, carrier = ca, r = cb/ca
        if abs(ca) <= abs(cb):
            carrier = cb
            r = ca / cb if cb != 0.0 else 0.0
            return False, r, carrier
        carrier = ca
        r = cb / ca if ca != 0.0 else 0.0
        return True, r, carrier

    swapE, rE, cE = pair(mE)
    swapO, rO, cO = pair(mO)
    # xs = cE*A' + cO*B'; scale the smaller carrier
    if abs(cO) <= abs(cE):
        swapF = False  # scale B' (Fa = Bp), carrier f = cE
        f = cE
        rB = cO / cE if cE != 0.0 else 0.0
    else:
        swapF = True   # scale A'
        f = cO
        rB = cE / cO if cO != 0.0 else 0.0

    scal = np.zeros((128, 8), np.float32)
    scal[:, 2] = rB
    scal[:, 3] = f
    scal[:, 5] = 0.25
    # ratios ride as float32 instruction immediates; round-trip through
    # float32 so the compile-key is exactly what the program computes
    rE32, rO32, rB32 = (np.float32(v) for v in (rE, rO, rB))
    return (swapE, swapO, swapF, float(rE32), float(rO32), float(rB32)), scal


def kernel(x, mask):
    import os

    global LAST_EXEC_NS, LAST_RESULTS

    x = np.asarray(x)
    mask = np.asarray(mask)
    assert x.shape == (B, H, W, C), x.shape
    in_dtype = x.dtype

    plan, scal = _mask_plan(mask)
    nc = _get_program(BPC, HO, NQ, WQ, C, plan)

    xv = np.ascontiguousarray(x, np.float32).reshape(B, HO, 2, NQ, WQ * C)

    in_maps = [
        {"x": xv[i * BPC : (i + 1) * BPC], "scal": scal} for i in range(N_CORES)
    ]

    trace = os.environ.get("KERNEL_TRACE", "0") == "1"
    res = run_bass_kernel_spmd(
        nc, in_maps, core_ids=list(range(N_CORES)), trace=trace
    )
    LAST_EXEC_NS = res.exec_time_ns
    LAST_RESULTS = res

    parts = [
        r["out"].reshape(BPC, HO, NQ, WQ // 2, C).reshape(BPC, HO, W // 2, C)
        for r in res.results
    ]
    full = np.concatenate(parts, axis=0)
    return full.astype(in_dtype, copy=False)


def _numpy_reference(x, mask):
    xr = x.reshape(x.shape[0], x.shape[1] // 2, 2, x.shape[2] // 2, 2, x.shape[3])
    x1 = xr.max(axis=(2, 4))
    x2 = xr.mean(axis=(2, 4))
    xs = np.einsum("bhiwjc,ij->bhwc", xr, mask)
    z = 1.0 / (1.0 + np.exp(-xs))
    return z * x1 + (1.0 - z) * x2


if __name__ == "__main__":
    # Small-scale CoreSim self-test (no hardware needed).
    from concourse.bass_interp import CoreSim

    rng = np.random.default_rng(0)
    for trial in range(4):
        bpc_s, nq_s, wq_s = 1, 1, 8
        h_s, w_s = 256, nq_s * wq_s
        xs_np = rng.standard_normal((bpc_s, h_s, w_s, C)).astype(np.float32)
        mask_np = (rng.standard_normal((2, 2)) * 0.5).astype(np.float32)

        plan_s, scal_s = _mask_plan(mask_np)
        nc = _build_program(bpc_s, 128, nq_s, wq_s, C, plan_s)
        sim = CoreSim(nc, trace=False)
        sim.tensor("x")[:] = xs_np.reshape(bpc_s, 128, 2, nq_s, wq_s * C)
        sim.tensor("scal")[:] = scal_s
        sim.simulate()
        got = (
            sim.tensor("out")
            .astype(np.float64)
            .reshape(bpc_s, 128, nq_s, wq_s // 2, C)
            .reshape(bpc_s, 128, w_s // 2, C)
        )
        want = _numpy_reference(xs_np.astype(np.float64), mask_np.astype(np.float64))
        err = np.abs(got - want)
        rel = err.max() / np.abs(want).max()
        print(f"trial {trial} plan={plan_s} mask={mask_np.reshape(-1)} "
              f"max abs {err.max():.2e} rel {rel:.2e}")
        assert rel < 5e-3, rel
    print("PASS")


# revision 4
# speedup vs baseline: 1.0260x; 1.0260x over previous
"""Gated max/avg 2x2 pooling kernel for Trainium2 (8 NeuronCores, SPMD) — v2.

Reference computation (per 2x2 window over [B, H, W, C], stride 2):
    x1 = max(window), x2 = mean(window)
    xs = sum_ij mask[i, j] * window[i, j]   (per channel)
    z  = sigmoid(xs)
    out = z * x1 + (1 - z) * x2

Sharding: pure data-parallel over batch (16 batches -> 2 per core).

Engine split (measured: DVE TT f16 ~0.59 ns/elem, TS ~0.35; ACT ~0.95
+ 0.4us/instr; GPSIMD ops stall concurrent DVE work so it is unused):
  ACT : f32->f16 cast (column-deinterleaving on the write, so every
        downstream slice is contiguous), sigmoid, q3 = rB*Fa, t4 = 0.25*s
  DVE : max tree (2 TT), sum tree (2 TT), xs pair-Horner (2 TS + 3 TT),
        blend (3 TT)
  DMA : f32 in (16 KiB contiguous runs per partition), f16 out (host
        upcasts to f32)

xs pair-Horner: Ap = rE*Ea + Eb, Bp = rO*Oa + Ob, xsp = rB*Fa + Fb,
z = sigmoid(f * xsp), with slice assignment (plan) chosen on the host so
every ratio has |r| <= 1; the carrier scale f rides the ACT sigmoid's
free pre-scale. Ratios are baked as instruction immediates (PTR-scalar
tensor_scalar ops can stall behind other engines' SBUF traffic).

Within a tile, the xs chain is emitted first so the DVE->ACT->DVE
sigmoid round-trip overlaps the independent max/sum trees.
"""

import numpy as np

import concourse.bacc as bacc
import concourse.mybir as mybir
import concourse.tile as tile
from concourse.bass_utils import run_bass_kernel_spmd

F32 = mybir.dt.float32
F16 = mybir.dt.float16

B, H, W, C = 16, 256, 256, 64
N_CORES = 8
BPC = B // N_CORES          # batches per core
HO = H // 2                 # 128 output rows = SBUF partitions
NQ = 4                      # w-quarters per row
WQ = W // NQ                # input w per macro-tile (64)

LAST_EXEC_NS = None
LAST_RESULTS = None

_PROGRAM_CACHE = {}


def _build_program(bpc, ho, nq, wq, ch, plan):
    """Build + compile the single-core Bass/Tile program (SPMD-shared).

    plan = (swapE, swapO, swapF, rE, rO, rB): slice assignment for the
    pair-Horner plus the ratio values, baked as instruction immediates
    (PTR-scalar tensor_scalar ops stall behind concurrent GPSIMD work;
    immediate-scalar ones do not).
    """
    from contextlib import ExitStack

    assert ho == 128, "partition dim must be 128"
    swapE, swapO, swapF, rE, rO, rB = plan
    fd_in = wq * ch            # free dim of an E/O tile (4096)
    wo = wq // 2               # output w per macro-tile
    fd_out = wo * ch           # free dim of output tile (2048)

    nc = bacc.Bacc(
        "TRN2",
        target_bir_lowering=False,
        debug=False,
        enable_asserts=True,
        num_devices=N_CORES,
    )

    x = nc.dram_tensor("x", [bpc, ho, 2, nq, fd_in], F32, kind="ExternalInput")
    scal = nc.dram_tensor("scal", [128, 8], F32, kind="ExternalInput")
    out = nc.dram_tensor("out", [bpc, ho, nq, fd_out], F16, kind="ExternalOutput")
    x_ap = x.ap()
    out_ap = out.ap()

    alu = mybir.AluOpType

    with tile.TileContext(nc) as tc, ExitStack() as ctx:
        pool_io = ctx.enter_context(tc.tile_pool(name="io", bufs=2))
        pool_y = ctx.enter_context(tc.tile_pool(name="y16", bufs=2))
        pool_big = ctx.enter_context(tc.tile_pool(name="big", bufs=1))
        pool_tmp = ctx.enter_context(tc.tile_pool(name="tmp", bufs=1))
        pool_gps = ctx.enter_context(tc.tile_pool(name="gps", bufs=2))
        pool_out = ctx.enter_context(tc.tile_pool(name="outp", bufs=2))
        pool_const = ctx.enter_context(tc.tile_pool(name="const", bufs=1))

        scal_t = pool_const.tile([128, 8], F32)
        nc.sync.dma_start(scal_t[:], scal.ap()[:])
        f_ap = scal_t[:, 3:4]
        rB_ap = scal_t[:, 2:3]
        zero_ap = scal_t[:, 4:5]
        quart_ap = scal_t[:, 5:6]  # 0.25

        def emit_load(b, q, w_lo, w_hi):
            """Stage 1: input DMA + ACT cast for one tile."""
            nw = w_hi - w_lo
            fde = nw * 2 * ch
            EO = pool_io.tile([128, 2 * fde], F32, tag="EO")
            src = x_ap[b, :, :, q, :].rearrange(
                "p r (w c) -> p r w c", c=2 * ch
            )[:, :, w_lo : w_lo + nw, :]
            nc.sync.dma_start(
                EO[:].rearrange("p (r w c) -> p r w c", r=2, c=2 * ch), src
            )
            Y = pool_y.tile([128, 2 * fde], F16, tag="Y")
            # cast + column-deinterleave: write order (r, e, w, c) while
            # reading DMA order (r, w, e, c); downstream slices contiguous.
            # One cast per row-parity keeps the APs within 3 free dims.
            Yw = Y[:].rearrange(
                "p (r e w c) -> p r w e c", r=2, e=2, c=ch
            )
            EOr = EO[:].rearrange(
                "p (r w e c) -> p r w e c", r=2, e=2, c=ch)
            nc.scalar.copy(Yw[:, 0], EOr[:, 0])
            nc.scalar.copy(Yw[:, 1], EOr[:, 1])
            return dict(b=b, q=q, w_lo=w_lo, nw=nw, fde=fde, fdo=nw * ch, Y=Y)

        def emit_compute(h):
            """Stage 2: DVE trees + sigmoid + GPS blend + output DMA."""
            b, q, w_lo, nw = h["b"], h["q"], h["w_lo"], h["nw"]
            fde, fdo, Y = h["fde"], h["fdo"], h["Y"]

            def tmp(tag, fd=fdo, dt=F16, pool=pool_tmp):
                t = pool.tile([128, fd], dt, tag=tag)
                return t

            Ef = Y[:, 0:fde]
            Of = Y[:, fde : 2 * fde]
            # deinterleaved layout: all four window slices are contiguous
            Ee, Eo = Y[:, 0:fdo], Y[:, fdo : 2 * fdo]
            Oe, Oo = Y[:, fde : fde + fdo], Y[:, fde + fdo : 2 * fde]

            def v(t):
                return t[:].rearrange("p (w c) -> p w c", c=ch)

            # xs chain first so the sigmoid's input is ready early; the
            # independent max/sum trees fill the ACT-latency windows
            Ea, Eb = (Eo, Ee) if swapE else (Ee, Eo)
            Oa, Ob = (Oo, Oe) if swapO else (Oe, Oo)
            q1 = tmp("q1")
            nc.vector.tensor_scalar_mul(q1[:], Ea, float(rE))
            Ap = tmp("Ap", pool=pool_gps)
            nc.vector.tensor_add(Ap[:], q1[:], Eb)
            q2 = tmp("q2")
            nc.vector.tensor_scalar_mul(q2[:], Oa, float(rO))
            Bp = tmp("Bp", pool=pool_gps)
            nc.vector.tensor_add(Bp[:], q2[:], Ob)
            Fa, Fb = (Ap, Bp) if swapF else (Bp, Ap)
            q3 = tmp("q3", pool=pool_gps)
            nc.scalar.mul(q3[:], Fa[:], rB_ap)

            # big vertical combines run while ACT computes q3
            M1 = tmp("M1", fd=fde, pool=pool_big)
            nc.vector.tensor_max(M1[:], Ef, Of)
            S1 = tmp("S1", fd=fde, pool=pool_big)
            nc.vector.tensor_add(S1[:], Ef, Of)

            xsp = tmp("xsp", pool=pool_gps)
            nc.vector.tensor_add(xsp[:], q3[:], Fb[:])
            z = tmp("z", pool=pool_gps)
            nc.scalar.activation(
                z[:], xsp[:],
                mybir.ActivationFunctionType.Sigmoid,
                bias=zero_ap, scale=f_ap,
            )

            # horizontal reduces + blend while ACT computes the sigmoid
            x1 = tmp("x1")
            nc.vector.tensor_max(x1[:], M1[:, 0:fdo], M1[:, fdo:fde])
            s = tmp("s", pool=pool_gps)
            nc.vector.tensor_add(s[:], S1[:, 0:fdo], S1[:, fdo:fde])
            t4 = tmp("t4", pool=pool_gps)
            nc.scalar.mul(t4[:], s[:], quart_ap)
            d = tmp("d", pool=pool_gps)
            nc.vector.tensor_sub(d[:], x1[:], t4[:])
            g = tmp("g", pool=pool_gps)
            nc.vector.tensor_mul(g[:], z[:], d[:])
            o = tmp("o", pool=pool_out)
            nc.vector.tensor_add(o[:], t4[:], g[:])

            dst = out_ap[b, :, q, :].rearrange("p (w c) -> p w c", c=ch)
            nc.sync.dma_start(
                dst[:, w_lo : w_lo + nw, :],
                o[:].rearrange("p (w c) -> p w c", c=ch),
            )

        wo_q = wq // 2  # output w-pairs per quarter
        n_macro = bpc * nq
        tiles = []
        for b in range(bpc):
            for qq in range(nq):
                first = not tiles
                last = b == bpc - 1 and qq == nq - 1
                if first:
                    tiles.append((b, qq, 0, wo_q // 4))
                    tiles.append((b, qq, wo_q // 4, wo_q // 2))
                    tiles.append((b, qq, wo_q // 2, wo_q))
                elif last:
                    tiles.append((b, qq, 0, wo_q // 2))
                    tiles.append((b, qq, wo_q // 2, wo_q))
                else:
                    tiles.append((b, qq, 0, wo_q))
        pending = emit_load(*tiles[0])
        for i in range(len(tiles)):
            nxt = emit_load(*tiles[i + 1]) if i + 1 < len(tiles) else None
            emit_compute(pending)
            pending = nxt

    nc.compile()
    return nc


def _get_program(bpc, ho, nq, wq, ch, plan):
    key = (bpc, ho, nq, wq, ch, plan)
    if key not in _PROGRAM_CACHE:
        _PROGRAM_CACHE[key] = _build_program(bpc, ho, nq, wq, ch, plan)
    return _PROGRAM_CACHE[key]


def _mask_plan(mask):
    """Derive (plan, scal[128,8]) so every ratio has |r| <= 1.

    xs = m00*Ee + m01*Eo + m10*Oe + m11*Oo
       = cE*(rE*Ea + Eb) + cO*(rO*Oa + Ob)
    with (Ea, Eb) = (Ee, Eo) or swapped so |rE| <= 1 (cE = the larger-|.|
    coefficient of the E pair), likewise the O pair. Final:
    xsp = rB*Fa + Fb with Fb the pair whose carrier |c| is larger;
    f = carrier of Fb, rB = other carrier / f.
    """
    m = np.asarray(mask, np.float64).reshape(-1)  # m00, m01, m10, m11
    mE = (m[0], m[1])
    mO = (m[2], m[3])

    def pair(coeffs):
        ca, cb = coeffs  # coeff of the 'even' slice, coeff of the 'odd' slice
        # swapped=False: A' = r*even + odd, carrier = cb (odd coeff), r = ca/cb
        # swapped=True:  A' = r*odd + even, carrier = ca, r = cb/ca
        if abs(ca) <= abs(cb):
            carrier = cb
            r = ca / cb if cb != 0.0 else 0.0
            return False, r, carrier
        carrier = ca
        r = cb / ca if ca != 0.0 else 0.0
        return True, r, carrier

    swapE, rE, cE = pair(mE)
    swapO, rO, cO = pair(mO)
    # xs = cE*A' + cO*B'; scale the smaller carrier
    if abs(cO) <= abs(cE):
        swapF = False  # scale B' (Fa = Bp), carrier f = cE
        f = cE
        rB = cO / cE if cE != 0.0 else 0.0
    else:
        swapF = True   # scale A'
        f = cO
        rB = cE / cO if cO != 0.0 else 0.0

    scal = np.zeros((128, 8), np.float32)
    scal[:, 2] = rB
    scal[:, 3] = f
    scal[:, 5] = 0.25
    # ratios ride as float32 instruction immediates; round-trip through
    # float32 so the compile-key is exactly what the program computes
    rE32, rO32, rB32 = (np.float32(v) for v in (rE, rO, rB))
    return (swapE, swapO, swapF, float(rE32), float(rO32), float(rB32)), scal


def kernel(x, mask):
    import os

    global LAST_EXEC_NS, LAST_RESULTS

    x = np.asarray(x)
    mask = np.asarray(mask)
    assert x.shape == (B, H, W, C), x.shape
    in_dtype = x.dtype

    plan, scal = _mask_plan(mask)
    nc = _get_program(BPC, HO, NQ, WQ, C, plan)

    xv = np.ascontiguousarray(x, np.float32).reshape(B, HO, 2, NQ, WQ * C)

    in_maps = [
        {"x": xv[i * BPC : (i + 1) * BPC], "scal": scal} for i in range(N_CORES)
    ]

    trace = os.environ.get("KERNEL_TRACE", "0") == "1"
    res = run_bass_kernel_spmd(
        nc, in_maps, core_ids=list(range(N_CORES)), trace=trace
    )
    LAST_EXEC_NS = res.exec_time_ns
    LAST_RESULTS = res

    parts = [
        r["out"].reshape(BPC, HO, NQ, WQ // 2, C).reshape(BPC, HO, W // 2, C)
        for r in res.results
    ]
    full = np.concatenate(parts, axis=0)
    return full.astype(in_dtype, copy=False)


def _numpy_reference(x, mask):
    xr = x.reshape(x.shape[0], x.shape[1] // 2, 2, x.shape[2] // 2, 2, x.shape[3])
    x1 = xr.max(axis=(2, 4))
    x2 = xr.mean(axis=(2, 4))
    xs = np.einsum("bhiwjc,ij->bhwc", xr, mask)
    z = 1.0 / (1.0 + np.exp(-xs))
    return z * x1 + (1.0 - z) * x2


if __name__ == "__main__":
    # Small-scale CoreSim self-test (no hardware needed).
    from concourse.bass_interp import CoreSim

    rng = np.random.default_rng(0)
    for trial in range(4):
        bpc_s, nq_s, wq_s = 1, 1, 8
        h_s, w_s = 256, nq_s * wq_s
        xs_np = rng.standard_normal((bpc_s, h_s, w_s, C)).astype(np.float32)
        mask_np = (rng.standard_normal((2, 2)) * 0.5).astype(np.float32)

        plan_s, scal_s = _mask_plan(mask_np)
        nc = _build_program(bpc_s, 128, nq_s, wq_s, C, plan_s)
        sim = CoreSim(nc, trace=False)
        sim.tensor("x")[:] = xs_np.reshape(bpc_s, 128, 2, nq_s, wq_s * C)
        sim.tensor("scal")[:] = scal_s
        sim.simulate()
        got = (
            sim.tensor("out")
            .astype(np.float64)
            .reshape(bpc_s, 128, nq_s, wq_s // 2, C)
            .reshape(bpc_s, 128, w_s // 2, C)
        )
        want = _numpy_reference(xs_np.astype(np.float64), mask_np.astype(np.float64))
        err = np.abs(got - want)
        rel = err.max() / np.abs(want).max()
        print(f"trial {trial} plan={plan_s} mask={mask_np.reshape(-1)} "
              f"max abs {err.max():.2e} rel {rel:.2e}")
        assert rel < 5e-3, rel
    print("PASS")
